# revision 7
# baseline (speedup 1.0000x reference)
"""Trainium2 Bass kernel for the CAM-threshold-subtract module.

Computation (per sample b):
    idx    = argmax(logits[b, :])                 # over 1000 classes
    cam    = interm[b, :, :, idx]                 # [7,7] gather
    t      = where(cam > 0.5, cam, 0)
    out[b] = vgg[b] - broadcast(t, [7,7,512])

Sharding: pure data parallel, batch 256 -> 8 cores x 32 samples.

Per-core memory traffic is dominated by vgg (3.2MB read) + out (3.2MB
write); interm is NOT streamed - only 49 floats per sample are fetched
with one indirect-DMA gather (32 descriptors), using a combined index
b*1000 + idx into a [32,1000,49]-strided logical view of interm.
"""

import numpy as np

M = 8          # cores
B = 32         # samples per core
S = 49         # spatial positions (7*7)
C = 512        # vgg channels
K = 1000       # classes
P = 128        # partitions
ROWS = B * S   # 1568 (b,pos) rows per core
NFULL = ROWS // P          # 12 full [128,512] tiles
REM = ROWS - NFULL * P     # 32 rows in the last tile
NT = NFULL + 1             # 13
THRESH = 0.5


def _build():
    import concourse.bacc as bacc
    import concourse.bass as bass
    import concourse.tile as tile
    from concourse import mybir

    nc = bacc.Bacc("TRN2", target_bir_lowering=False, debug=False)
    vgg = nc.dram_tensor("vgg", [ROWS, C], mybir.dt.float32, kind="ExternalInput")
    # interm is pre-transposed on host to [B, K, S] so each CAM row
    # (one channel's 49 spatial values) is contiguous for the row-gather.
    interm = nc.dram_tensor("interm", [B, K, S], mybir.dt.float32, kind="ExternalInput")
    logits = nc.dram_tensor("logits", [B, K], mybir.dt.float32, kind="ExternalInput")
    out = nc.dram_tensor("out", [ROWS, C], mybir.dt.float32, kind="ExternalOutput")

    with tile.TileContext(nc) as tc:
        with (
            tc.tile_pool(name="big", bufs=NT) as big,
            tc.tile_pool(name="small", bufs=1) as small,
            tc.tile_pool(name="dram", bufs=1, space="DRAM") as dpool,
        ):
            # ---- per-sample argmax over class logits ----
            lg = small.tile([B, K], mybir.dt.float32)
            nc.sync.dma_start(out=lg[:], in_=logits.ap()[:, :])
            mx = small.tile([B, 8], mybir.dt.float32)
            nc.vector.max(mx[:], lg[:])
            mi = small.tile([B, 8], mybir.dt.uint32)
            nc.vector.max_index(mi[:], mx[:], lg[:])

            # combined row index into interm viewed [B*K, S]: b*1000 + idx_b
            base = small.tile([B, 1], mybir.dt.uint32)
            nc.gpsimd.iota(base[:], [[1, 1]], base=0, channel_multiplier=K)
            comb = small.tile([B, 1], mybir.dt.uint32)
            nc.vector.tensor_tensor(
                out=comb[:], in0=mi[:, 0:1], in1=base[:], op=mybir.AluOpType.add
            )

            # ---- row-gather the selected CAM: cam[b, :] = interm[b, idx_b, :] ----
            # one descriptor per sample, 49 contiguous floats each
            cam = small.tile([B, S], mybir.dt.float32)
            nc.gpsimd.indirect_dma_start(
                out=cam[:],
                out_offset=None,
                in_=interm.ap().rearrange("b k s -> (b k) s"),
                in_offset=bass.IndirectOffsetOnAxis(ap=comb[:, 0:1], axis=0),
            )

            # ---- threshold: t = cam * (cam > 0.5) ----
            mask = small.tile([B, S], mybir.dt.float32)
            nc.vector.tensor_scalar(
                out=mask[:], in0=cam[:], scalar1=THRESH, scalar2=None,
                op0=mybir.AluOpType.is_gt,
            )
            tt = small.tile([B, S], mybir.dt.float32)
            nc.vector.tensor_tensor(
                out=tt[:], in0=cam[:], in1=mask[:], op=mybir.AluOpType.mult
            )

            # ---- refold t from [32 samples, 49] to [128 rows, 13 tiles] ----
            # (row g = b*49+pos; tile k holds rows 128k..128k+127; needs a
            # partition-dim reshape, so bounce through DRAM)
            td = dpool.tile([NT, P], mybir.dt.float32)  # flat [1664]
            nc.sync.dma_start(
                out=td[:].flatten()[0:ROWS].rearrange("(b s) -> b s", b=B),
                in_=tt[:],
            )
            # zero the 96-element pad tail so the t_all reload reads no junk
            zpad = small.tile([1, P - REM], mybir.dt.float32)
            nc.vector.memset(zpad[:], 0.0)
            nc.sync.dma_start(out=td[NFULL : NFULL + 1, REM:P], in_=zpad[:])
            t_all = small.tile([P, NT], mybir.dt.float32)
            nc.sync.dma_start(out=t_all[:], in_=td[:].transpose([1, 0]))

            # ---- main stream: out = vgg - t (per-row scalar broadcast) ----
            for k in range(NT):
                rows = P if k < NFULL else REM
                vt = big.tile([P, C], mybir.dt.float32, tag="vt")
                nc.sync.dma_start(
                    out=vt[:rows, :], in_=vgg.ap()[k * P : k * P + rows, :]
                )
                nc.vector.tensor_scalar(
                    out=vt[:rows, :], in0=vt[:rows, :],
                    scalar1=t_all[:rows, k : k + 1], scalar2=None,
                    op0=mybir.AluOpType.subtract,
                )
                nc.scalar.dma_start(
                    out=out.ap()[k * P : k * P + rows, :], in_=vt[:rows, :]
                )
    nc.compile()
    return nc


_NC = None


def _get_nc():
    global _NC
    if _NC is None:
        _NC = _build()
    return _NC


def _shard(vgg_end, interm, branchA_end):
    in_maps = []
    for i in range(M):
        sl = slice(i * B, (i + 1) * B)
        in_maps.append(
            {
                "vgg": np.ascontiguousarray(vgg_end[sl], dtype=np.float32).reshape(ROWS, C),
                "interm": np.ascontiguousarray(
                    np.asarray(interm[sl], dtype=np.float32).reshape(B, S, K).transpose(0, 2, 1)
                ),
                "logits": np.ascontiguousarray(branchA_end[sl], dtype=np.float32),
            }
        )
    return in_maps


def kernel(vgg_end, interm, branchA_end, _trace=False):
    from concourse.bass_utils import run_bass_kernel_spmd

    nc = _get_nc()
    in_maps = _shard(np.asarray(vgg_end), np.asarray(interm), np.asarray(branchA_end))
    res = run_bass_kernel_spmd(nc, in_maps, core_ids=list(range(M)), trace=_trace)
    full = np.concatenate(
        [r["out"].reshape(B, 7, 7, C) for r in res.results], axis=0
    )
    if _trace:
        return full, res
    return full


# revision 8
# speedup vs baseline: 22.3928x; 22.3928x over previous
"""Trainium2 Bass kernel for the CAM-threshold-subtract module.

Computation (per sample b):
    idx    = argmax(logits[b, :])                 # over 1000 classes
    cam    = interm[b, :, :, idx]                 # [7,7] gather
    t      = where(cam > 0.5, cam, 0)
    out[b] = vgg[b] - broadcast(t, [7,7,512])

Sharding: pure data parallel, batch 256 -> 8 cores x 32 samples.

Per-core memory traffic is dominated by vgg (3.2MB read) + out (3.2MB
write); interm is NOT streamed - only 49 floats per sample are fetched
with one indirect-DMA gather (32 descriptors), using a combined index
b*1000 + idx into a [32,1000,49]-strided logical view of interm.
"""

import numpy as np

M = 8          # cores
B = 32         # samples per core
S = 49         # spatial positions (7*7)
C = 512        # vgg channels
K = 1000       # classes
P = 128        # partitions
ROWS = B * S   # 1568 (b,pos) rows per core
NFULL = ROWS // P          # 12 full [128,512] tiles
REM = ROWS - NFULL * P     # 32 rows in the last tile
NT = NFULL + 1             # 13
THRESH = 0.5


def _build(loop_n=None):
    import contextlib

    import concourse.bacc as bacc
    import concourse.bass as bass
    import concourse.tile as tile
    from concourse import mybir

    nc = bacc.Bacc("TRN2", target_bir_lowering=False, debug=False)
    vgg = nc.dram_tensor("vgg", [ROWS, C], mybir.dt.float32, kind="ExternalInput")
    # interm is pre-transposed on host to [B, K, S] so each CAM row
    # (one channel's 49 spatial values) is contiguous for the row-gather.
    interm = nc.dram_tensor("interm", [B, K, S], mybir.dt.float32, kind="ExternalInput")
    logits = nc.dram_tensor("logits", [B, K], mybir.dt.float32, kind="ExternalInput")
    out = nc.dram_tensor("out", [ROWS, C], mybir.dt.float32, kind="ExternalOutput")

    with tile.TileContext(nc) as tc:
        with (
            tc.tile_pool(name="big", bufs=NT) as big,
            tc.tile_pool(name="small", bufs=1) as small,
            tc.tile_pool(name="dram", bufs=1, space="DRAM") as dpool,
            tc.For_i(0, loop_n) if loop_n else contextlib.nullcontext(),
        ):
            # ---- per-sample argmax over class logits ----
            lg = small.tile([B, K], mybir.dt.float32)
            nc.sync.dma_start(out=lg[:], in_=logits.ap()[:, :])
            mx = small.tile([B, 8], mybir.dt.float32)
            nc.vector.max(mx[:], lg[:])
            mi = small.tile([B, 8], mybir.dt.uint32)
            nc.vector.max_index(mi[:], mx[:], lg[:])

            # combined row index into interm viewed [B*K, S]: b*1000 + idx_b
            base = small.tile([B, 1], mybir.dt.uint32)
            nc.gpsimd.iota(base[:], [[1, 1]], base=0, channel_multiplier=K)
            comb = small.tile([B, 1], mybir.dt.uint32)
            nc.vector.tensor_tensor(
                out=comb[:], in0=mi[:, 0:1], in1=base[:], op=mybir.AluOpType.add
            )

            # ---- row-gather the selected CAM: cam[b, :] = interm[b, idx_b, :] ----
            # one descriptor per sample, 49 contiguous floats each
            cam = small.tile([B, S], mybir.dt.float32)
            nc.gpsimd.indirect_dma_start(
                out=cam[:],
                out_offset=None,
                in_=interm.ap().rearrange("b k s -> (b k) s"),
                in_offset=bass.IndirectOffsetOnAxis(ap=comb[:, 0:1], axis=0),
            )

            # ---- threshold: t = cam * (cam > 0.5) ----
            mask = small.tile([B, S], mybir.dt.float32)
            nc.vector.tensor_scalar(
                out=mask[:], in0=cam[:], scalar1=THRESH, scalar2=None,
                op0=mybir.AluOpType.is_gt,
            )
            tt = small.tile([B, S], mybir.dt.float32)
            nc.vector.tensor_tensor(
                out=tt[:], in0=cam[:], in1=mask[:], op=mybir.AluOpType.mult
            )

            # ---- refold t from [32 samples, 49] to [128 rows, 13 tiles] ----
            # (row g = b*49+pos; tile k holds rows 128k..128k+127; needs a
            # partition-dim reshape, so bounce through DRAM)
            td = dpool.tile([NT, P], mybir.dt.float32)  # flat [1664]
            nc.sync.dma_start(
                out=td[:].flatten()[0:ROWS].rearrange("(b s) -> b s", b=B),
                in_=tt[:],
            )
            # zero the 96-element pad tail so the t_all reload reads no junk
            zpad = small.tile([1, P - REM], mybir.dt.float32)
            nc.vector.memset(zpad[:], 0.0)
            nc.sync.dma_start(out=td[NFULL : NFULL + 1, REM:P], in_=zpad[:])
            t_all = small.tile([P, NT], mybir.dt.float32)
            nc.sync.dma_start(out=t_all[:], in_=td[:].transpose([1, 0]))

            # ---- main stream: out = vgg - t (per-row scalar broadcast) ----
            for k in range(NT):
                rows = P if k < NFULL else REM
                vt = big.tile([P, C], mybir.dt.float32, tag="vt")
                nc.sync.dma_start(
                    out=vt[:rows, :], in_=vgg.ap()[k * P : k * P + rows, :]
                )
                nc.vector.tensor_scalar(
                    out=vt[:rows, :], in0=vt[:rows, :],
                    scalar1=t_all[:rows, k : k + 1], scalar2=None,
                    op0=mybir.AluOpType.subtract,
                )
                nc.scalar.dma_start(
                    out=out.ap()[k * P : k * P + rows, :], in_=vt[:rows, :]
                )
    nc.compile()
    return nc


_NC = None


def _get_nc():
    global _NC
    if _NC is None:
        _NC = _build()
    return _NC


def _shard(vgg_end, interm, branchA_end):
    in_maps = []
    for i in range(M):
        sl = slice(i * B, (i + 1) * B)
        in_maps.append(
            {
                "vgg": np.ascontiguousarray(vgg_end[sl], dtype=np.float32).reshape(ROWS, C),
                "interm": np.ascontiguousarray(
                    np.asarray(interm[sl], dtype=np.float32).reshape(B, S, K).transpose(0, 2, 1)
                ),
                "logits": np.ascontiguousarray(branchA_end[sl], dtype=np.float32),
            }
        )
    return in_maps


def kernel(vgg_end, interm, branchA_end, _trace=False):
    from concourse.bass_utils import run_bass_kernel_spmd

    nc = _get_nc()
    in_maps = _shard(np.asarray(vgg_end), np.asarray(interm), np.asarray(branchA_end))
    res = run_bass_kernel_spmd(nc, in_maps, core_ids=list(range(M)), trace=_trace)
    full = np.concatenate(
        [r["out"].reshape(B, 7, 7, C) for r in res.results], axis=0
    )
    if _trace:
        return full, res
    return full
